# revision 12
# baseline (speedup 1.0000x reference)
import os
import sys
import math

sys.path.insert(0, "/opt/trn_rl_repo")

import numpy as np
import ml_dtypes

import concourse.bass as bass
import concourse.mybir as mybir
import concourse.tile as tile
from concourse import bacc
from concourse import library_config
from concourse.bass_utils import run_bass_kernel_spmd

P = 128
NCORES = 8
B = 8
BF16 = mybir.dt.bfloat16
F32 = mybir.dt.float32
I16 = mybir.dt.int16
AF = mybir.ActivationFunctionType
ALU = mybir.AluOpType

CFG_FULL = dict(N_G=50000, E_G=800000, N_S=10000, E_S=160000, NF=64)

Y3PAD = 256



def _assign_nodes(dst, n_nodes):
    import heapq
    deg = np.bincount(dst, minlength=n_nodes)
    slots = math.ceil(n_nodes / (NCORES * P))
    nbins = NCORES * slots
    order = np.argsort(-deg, kind="stable")
    heap = [(0, 0, b) for b in range(nbins)]
    heapq.heapify(heap)
    bin_of = np.empty(n_nodes, np.int64)
    bin_fill = np.zeros(nbins, np.int64)
    pos_of = np.empty(n_nodes, np.int64)
    for n in order:
        while True:
            load, cnt, b = heapq.heappop(heap)
            if bin_fill[b] < P:
                break
        bin_of[n] = b
        pos_of[n] = bin_fill[b]
        bin_fill[b] += 1
        if bin_fill[b] < P:
            heapq.heappush(heap, (load + int(deg[n]), cnt + 1, b))
    bin_load = np.zeros(nbins, np.int64)
    np.add.at(bin_load, bin_of[dst], 1)
    core_of_bin = np.arange(nbins) % NCORES
    slot_of_bin = np.empty(nbins, np.int64)
    for c in range(NCORES):
        bins_c = np.where(core_of_bin == c)[0]
        order_c = bins_c[np.argsort(-bin_load[bins_c], kind="stable")]
        slot_of_bin[order_c] = np.arange(slots)
    core_of = core_of_bin[bin_of]
    slot_of = slot_of_bin[bin_of]
    perm = core_of * (slots * P) + slot_of * P + pos_of
    return perm, core_of, slot_of, pos_of, slots


def _build_edges(src, dst, perm, core_of, slot_of, pos_of, slots, parts):
    S = slots * P
    nparts = len(parts)
    ecore = core_of[dst]
    eslot = slot_of[dst]
    psrc = perm[src]
    pc = psrc // S
    pr = psrc % S
    ps_slot = pr // P
    part_of = np.zeros(len(src), np.int64)
    rowid = np.empty(len(src), np.int64)
    for pi, (s0, s1) in enumerate(parts):
        m = (ps_slot >= s0) & (ps_slot < s1)
        part_of[m] = pi
        Sp = (s1 - s0) * P
        rowid[m] = pc[m] * Sp + (pr[m] - s0 * P)
        assert Sp * NCORES <= 32768
    cnt = np.zeros((NCORES, slots, nparts), np.int64)
    np.add.at(cnt, (ecore, eslot, part_of), 1)
    nch_p = np.ceil(cnt.max(axis=0) / P).astype(np.int64)
    nch_p[:, 0] = np.maximum(1, nch_p[:, 0])
    nch = nch_p.sum(axis=1)
    tc = int(nch.sum())
    pstart = np.zeros((slots, nparts), np.int64)
    col = 0
    for s in range(slots):
        for pi in range(nparts):
            pstart[s, pi] = col
            col += nch_p[s, pi]
    src_arr = np.zeros((NCORES, P, tc), np.int32)
    dstl_arr = np.full((NCORES, P, tc), 255.0, np.float32)
    idx16 = np.zeros((NCORES, 16, tc * 8), np.int16)
    order = np.lexsort((part_of, eslot, ecore))
    ec, esl, ep = ecore[order], eslot[order], part_of[order]
    psrc_o = psrc[order].astype(np.int32)
    rid_o = rowid[order].astype(np.int16)
    pdst = pos_of[dst][order].astype(np.float32)
    key = (ec * slots + esl) * nparts + ep
    bounds = np.searchsorted(key, np.arange(NCORES * slots * nparts + 1))
    for c in range(NCORES):
        for s in range(slots):
            for pi in range(nparts):
                k = (c * slots + s) * nparts + pi
                a, b = bounds[k], bounds[k + 1]
                n = b - a
                ncol = int(nch_p[s, pi])
                if ncol == 0:
                    assert n == 0
                    continue
                c0 = int(pstart[s, pi])
                buf_s = np.zeros(P * ncol, np.int32)
                buf_d = np.full(P * ncol, 255.0, np.float32)
                buf_i = np.zeros(P * ncol, np.int16)
                buf_s[:n] = psrc_o[a:b]
                buf_d[:n] = pdst[a:b]
                buf_i[:n] = rid_o[a:b]
                src_arr[c, :, c0:c0 + ncol] = buf_s.reshape(ncol, P).T
                dstl_arr[c, :, c0:c0 + ncol] = buf_d.reshape(ncol, P).T
                idx16[c, :, c0 * 8:(c0 + ncol) * 8] = \
                    buf_i.reshape(ncol * 8, 16).T
    return src_arr, dstl_arr, idx16, nch_p, pstart, nch.astype(int), tc


def _prep_branch(x, edge_index, batch, n_nodes):
    src = np.asarray(edge_index[0], np.int64)
    dst = np.asarray(edge_index[1], np.int64)
    perm, core_of, slot_of, pos_of, slots = _assign_nodes(dst, n_nodes)
    max_slots_per_part = 32768 // (NCORES * P)
    nparts = math.ceil(slots / max_slots_per_part)
    bnds = np.linspace(0, slots, nparts + 1).astype(int)
    parts = [(int(bnds[i]), int(bnds[i + 1])) for i in range(nparts)]
    src_arr, dstl_arr, idx16, nch_p, pstart, nch, tc = _build_edges(
        src, dst, perm, core_of, slot_of, pos_of, slots, parts)
    npad = NCORES * slots * P
    S = slots * P
    nf = x.shape[1]
    x_full = np.zeros((npad, nf), np.float32)
    x_full[perm] = np.asarray(x, np.float32)
    xT = np.stack([x_full[c * S:(c + 1) * S].T.copy() for c in range(NCORES)])
    bf = ml_dtypes.bfloat16
    xf16 = x_full.astype(bf)
    e1 = np.stack([xf16[src_arr[c]].reshape(P, tc * nf) for c in range(NCORES)])
    ohB = np.zeros((NCORES, P, slots * B), np.float32)
    bvec = np.asarray(batch, np.int64)
    pid = perm
    c_all, r_all = np.divmod(pid, S)
    s_all, p_all = np.divmod(r_all, P)
    for n in range(n_nodes):
        ohB[c_all[n], p_all[n], s_all[n] * B + int(bvec[n])] = 1.0
    cntb = np.bincount(bvec, minlength=B).astype(np.float32)
    recip = 1.0 / np.maximum(cntb, 1.0)
    corr = (cntb > 0).astype(np.float32)
    return dict(perm=perm, slots=slots, S=S, npad=npad, nch=nch, tc=tc,
                parts=parts, nch_p=nch_p, pstart=pstart,
                src_arr=src_arr, dstl_arr=dstl_arr, idx16=idx16,
                x_full=x_full, xT=xT, e1=e1, ohB=ohB, recip=recip, corr=corr)


def _pack_wt(w):
    return np.ascontiguousarray(np.asarray(w, np.float32).T)


def _pack_bias(bvec, nchunks):
    out = np.zeros((P, nchunks), np.float32)
    b = np.asarray(bvec, np.float32)
    for j in range(nchunks):
        seg = b[j * P:(j + 1) * P]
        out[:len(seg), j] = seg
    return out


def host_prep(inputs, cfg):
    g = _prep_branch(inputs["graph_x"], inputs["graph_edge_index"],
                     inputs["graph_batch"], cfg["N_G"])
    s = _prep_branch(inputs["subgraph_x"], inputs["subgraph_edge_index"],
                     inputs["subgraph_batch"], cfg["N_S"])
    NF = cfg["NF"]
    meta = dict(g=g, s=s, NF=NF)

    bf = ml_dtypes.bfloat16
    common = {}
    common["iota"] = np.broadcast_to(
        np.arange(P, dtype=np.float32), (P, P)).copy()
    common["ident"] = np.eye(P, dtype=np.float32)

    dims = [(2 * NF, NF), (4 * NF, 2 * NF), (3 * NF, 4 * NF)]
    meta["dims"] = dims
    for pre in ("g", "s"):
        for li, (o, c) in enumerate(dims, start=1):
            common[f"{pre}W{li}rT"] = _pack_wt(inputs[f"{pre}W{li}r"]).astype(bf)
            common[f"{pre}W{li}nT"] = _pack_wt(inputs[f"{pre}W{li}n"]).astype(bf)
            common[f"{pre}B{li}"] = _pack_bias(inputs[f"{pre}B{li}"],
                                               math.ceil(o / P))
        b3 = np.asarray(inputs[f"{pre}B3"], np.float32)
        common[f"{pre}B3nm"] = np.broadcast_to(b3, (P, 3 * NF)).copy()
    O3 = 3 * NF
    zmap = np.full(5 * P, -1, np.int64)
    zmap[0:P] = np.arange(0, P)
    zmap[P:P + (O3 - P)] = np.arange(P, O3)
    zmap[2 * P:3 * P] = O3 + np.arange(0, P)
    zmap[3 * P:3 * P + (O3 - P)] = O3 + np.arange(P, O3)
    zmap[4 * P:4 * P + NF] = 2 * O3 + np.arange(NF)
    l1W = np.asarray(inputs["l1W"], np.float32)
    l1WT = np.zeros((5 * P, 600), np.float32)
    valid = zmap >= 0
    l1WT[valid] = l1W[:, zmap[valid]].T
    H1, H2 = 600, 256
    M1 = math.ceil(H1 / P)
    l2W = np.asarray(inputs["l2W"], np.float32)
    l2WT = np.zeros((M1 * P, H2), np.float32)
    l2WT[:H1] = l2W.T
    l3W = np.asarray(inputs["l3W"], np.float32)
    l3WT = np.ascontiguousarray(l3W.T)

    def pack_k(wt, kchunks, width):
        out = np.zeros((P, kchunks * width), np.float32)
        for k in range(kchunks):
            seg = wt[k * P:(k + 1) * P]
            out[:seg.shape[0], k * width:k * width + width] = seg
        return out

    common["l1WT"] = pack_k(l1WT, 5, 600)
    common["l2WT"] = pack_k(l2WT, M1, H2)
    common["l3WT"] = pack_k(l3WT, 2, NF)
    common["l1b"] = _pack_bias(inputs["l1b"], M1)
    common["l2b"] = _pack_bias(inputs["l2b"], 2)
    common["l3b"] = _pack_bias(inputs["l3b"], 1)
    common["pointT"] = np.ascontiguousarray(
        np.asarray(inputs["point"], np.float32).T)

    in_maps = []
    for c in range(NCORES):
        m = dict(common)
        m["g_dstl"] = g["dstl_arr"][c]
        m["s_dstl"] = s["dstl_arr"][c]
        m["g_idx"] = np.tile(g["idx16"][c], (8, 1))
        m["s_idx"] = np.tile(s["idx16"][c], (8, 1))
        m["g_e1"] = g["e1"][c]
        m["s_e1"] = s["e1"][c]
        m["xg1T"] = g["xT"][c].astype(bf)
        m["xs1T"] = s["xT"][c].astype(bf)
        m["g_ohB"] = g["ohB"][c]
        m["s_ohB"] = s["ohB"][c]
        m["g_recip"] = np.broadcast_to(g["recip"], (P, B)).astype(
            np.float32).copy()
        m["s_recip"] = np.broadcast_to(s["recip"], (P, B)).astype(
            np.float32).copy()
        m["g_corr"] = np.broadcast_to(g["corr"], (P, B)).astype(
            np.float32).copy()
        m["s_corr"] = np.broadcast_to(s["corr"], (P, B)).astype(
            np.float32).copy()
        in_maps.append(m)
    return meta, in_maps



def _ap3(t_ap, mid_count):
    return bass.AP(t_ap.tensor, t_ap.offset,
                   [list(t_ap.ap[0]), [0, mid_count], list(t_ap.ap[1])])


def build_program(meta, debug=False):
    NF = meta["NF"]
    dims = meta["dims"]
    g, s = meta["g"], meta["s"]
    O3 = 3 * NF

    nc = bacc.Bacc(None, target_bir_lowering=False, debug=False,
                   num_swdge_queues=4)

    def din(name, shape, dtype):
        return nc.dram_tensor(name, list(shape), dtype, kind="ExternalInput")

    xg1T = din("xg1T", [NF, g["S"]], BF16)
    xs1T = din("xs1T", [NF, s["S"]], BF16)
    g_dstl = din("g_dstl", [P, g["tc"]], F32)
    s_dstl = din("s_dstl", [P, s["tc"]], F32)
    g_idx = din("g_idx", [P, g["tc"] * 8], I16)
    s_idx = din("s_idx", [P, s["tc"] * 8], I16)
    g_e1 = din("g_e1", [P, g["tc"] * NF], BF16)
    s_e1 = din("s_e1", [P, s["tc"] * NF], BF16)
    iota_in = din("iota", [P, P], F32)
    ident_in = din("ident", [P, P], F32)
    wts = {}
    for pre in ("g", "s"):
        for li, (o, c) in enumerate(dims, start=1):
            wts[f"{pre}W{li}rT"] = din(f"{pre}W{li}rT", [c, o], BF16)
            wts[f"{pre}W{li}nT"] = din(f"{pre}W{li}nT", [c, o], BF16)
            wts[f"{pre}B{li}"] = din(f"{pre}B{li}", [P, math.ceil(o / P)], F32)
        wts[f"{pre}B3nm"] = din(f"{pre}B3nm", [P, O3], F32)
    l1WT = din("l1WT", [P, 5 * 600], F32)
    l2WT = din("l2WT", [P, 5 * 256], F32)
    l3WT = din("l3WT", [P, 2 * NF], F32)
    l1b = din("l1b", [P, 5], F32)
    l2b = din("l2b", [P, 2], F32)
    l3b = din("l3b", [P, 1], F32)
    pointT = din("pointT", [NF, B], F32)
    g_ohB = din("g_ohB", [P, g["slots"] * B], F32)
    s_ohB = din("s_ohB", [P, s["slots"] * B], F32)
    g_recip = din("g_recip", [P, B], F32)
    s_recip = din("s_recip", [P, B], F32)
    g_corr = din("g_corr", [P, B], F32)
    s_corr = din("s_corr", [P, B], F32)

    out_ext = nc.dram_tensor("out", [B, NF], F32, kind="ExternalOutput")

    with tile.TileContext(nc) as tc:
        with tc.tile_pool(name="const", bufs=1) as cp, \
             tc.tile_pool(name="gat", bufs=8) as gat_p, \
             tc.tile_pool(name="e1p", bufs=3) as e1_p, \
             tc.tile_pool(name="oh", bufs=3) as oh_p, \
             tc.tile_pool(name="evac", bufs=3) as ev_p, \
             tc.tile_pool(name="elu", bufs=2) as elu_p, \
             tc.tile_pool(name="stage", bufs=3) as st_p, \
             tc.tile_pool(name="psA", bufs=2, space="PSUM") as psA, \
             tc.tile_pool(name="psB", bufs=2, space="PSUM") as psB, \
             tc.tile_pool(name="psT", bufs=2, space="PSUM") as psT, \
             tc.tile_pool(name="psPool", bufs=1, space="PSUM") as psPool, \
             tc.tile_pool(name="dram", bufs=1, space="DRAM") as dram:

            nc.gpsimd.load_library(library_config.mlp)

            def load_const(name, src_t, shape, dtype):
                t = cp.tile(list(shape), dtype, tag=name)
                nc.sync.dma_start(out=t[:], in_=src_t[:])
                return t

            gdstl_t = load_const("g_dstl", g_dstl, [P, g["tc"]], F32)
            sdstl_t = load_const("s_dstl", s_dstl, [P, s["tc"]], F32)
            gidx_t = load_const("g_idx", g_idx, [P, g["tc"] * 8], I16)
            sidx_t = load_const("s_idx", s_idx, [P, s["tc"] * 8], I16)
            iota_t = load_const("iota", iota_in, [P, P], F32)
            ident_t = load_const("ident", ident_in, [P, P], F32)
            w_t = {}
            for pre in ("g", "s"):
                for li, (o, c) in enumerate(dims, start=1):
                    for rn in ("r", "n"):
                        nm = f"{pre}W{li}{rn}T"
                        kch = math.ceil(c / P)
                        t = cp.tile([P, kch * o], BF16, tag=nm)
                        src_w = wts[nm]
                        if c < P:
                            nc.sync.dma_start(out=t[:c, :o], in_=src_w[:])
                        else:
                            nc.sync.dma_start(
                                out=t[:].rearrange("p (k o) -> p k o", k=kch),
                                in_=src_w[:].rearrange("(k p) o -> p k o",
                                                       p=P))
                        w_t[nm] = t
                    nm = f"{pre}B{li}"
                    w_t[nm] = load_const(nm, wts[nm],
                                         [P, math.ceil(o / P)], F32)
                nm = f"{pre}B3nm"
                w_t[nm] = load_const(nm, wts[nm], [P, O3], F32)
            l1w_t = load_const("l1WT", l1WT, [P, 5 * 600], F32)
            l2w_t = load_const("l2WT", l2WT, [P, 5 * 256], F32)
            l3w_t = load_const("l3WT", l3WT, [P, 2 * NF], F32)
            l1b_t = load_const("l1b", l1b, [P, 5], F32)
            l2b_t = load_const("l2b", l2b, [P, 2], F32)
            l3b_t = load_const("l3b", l3b, [P, 1], F32)
            pointT_t = load_const("pointT", pointT, [NF, B], F32)
            gohB_t = load_const("g_ohB", g_ohB, [P, g["slots"] * B], F32)
            sohB_t = load_const("s_ohB", s_ohB, [P, s["slots"] * B], F32)
            grec_t = load_const("g_recip", g_recip, [P, B], F32)
            srec_t = load_const("s_recip", s_recip, [P, B], F32)
            gcorr_t = load_const("g_corr", g_corr, [P, B], F32)
            scorr_t = load_const("s_corr", s_corr, [P, B], F32)
            neg1_t = cp.tile([P, 1], F32, tag="neg1", name="neg1")
            nc.vector.memset(neg1_t[:], -1.0)
            ones_t = cp.tile([P, P], F32, tag="ones", name="ones")
            nc.vector.memset(ones_t[:], 1.0)

            xT_store = {}
            for pre, br in (("g", g), ("s", s)):
                for ab in "AB":
                    xT_store[pre + ab] = cp.tile(
                        [P, 2 * br["slots"] * P], BF16,
                        tag=f"xT{pre}{ab}", name=f"xT{pre}{ab}")
            nc.sync.dma_start(out=xT_store["gA"][:NF, :g["S"]], in_=xg1T[:])
            nc.sync.dma_start(out=xT_store["sA"][:NF, :s["S"]], in_=xs1T[:])

            def branch_dram(pre, br):
                d = {}
                d["x2_shard"] = dram.tile([br["S"], 2 * NF], BF16,
                                          tag=f"x2sh_{pre}",
                                          name=f"x2sh_{pre}")
                d["y3_shard"] = dram.tile([br["S"], Y3PAD], BF16,
                                          tag=f"y3sh_{pre}",
                                          name=f"y3sh_{pre}")
                d["x2_full"] = []
                d["y3_full"] = []
                for pi, (s0, s1) in enumerate(br["parts"]):
                    rows = (s1 - s0) * P * NCORES
                    d["x2_full"].append(dram.tile(
                        [rows, 2 * NF], BF16, tag=f"x2f_{pre}{pi}",
                        name=f"x2f_{pre}{pi}", addr_space="Shared"))
                    d["y3_full"].append(dram.tile(
                        [rows, Y3PAD], BF16, tag=f"y3f_{pre}{pi}",
                        name=f"y3f_{pre}{pi}", addr_space="Shared"))
                return d

            gd = branch_dram("g", g)
            sd = branch_dram("s", s)
            ar_in = dram.tile([P, 4 * B], F32, tag="ar_in")
            ar_out = dram.tile([P, 4 * B], F32, tag="ar_out")

            poolacc = cp.tile([P, 2 * O3], F32, tag="poolacc")
            nc.vector.memset(poolacc[:], 0.0)

            dbg_x4 = None
            if debug:
                dbg_x4 = nc.dram_tensor("dbg_x4", [g["S"], O3], F32,
                                        kind="ExternalOutput")

            def ag_part(shard_t, full_t, s0, s1):
                nc.gpsimd.collective_compute(
                    "AllGather", ALU.bypass,
                    replica_groups=[list(range(NCORES))],
                    ins=[shard_t[s0 * P:s1 * P, :].opt()],
                    outs=[full_t[:].opt()])

            def make_oh(dstl_t, c0, n_j):
                oh_t = oh_p.tile([P, n_j * P], BF16, tag="oh")
                d_ap = dstl_t[:, c0:c0 + n_j].to_broadcast([P, n_j, P])
                i_ap = _ap3(iota_t[:], n_j)
                nc.vector.tensor_tensor(
                    out=oh_t[:].rearrange("p (k q) -> p k q", k=n_j),
                    in0=d_ap, in1=i_ap, op=ALU.is_equal)
                return oh_t

            GMAX = 8
            q_rr = [0]

            def emit_gathers_parts(br, idx_t, fulls, slot, elem,
                                   which_parts):
                tiles = []
                joff = 0
                for pi in range(len(br["parts"])):
                    n = int(br["nch_p"][slot, pi])
                    if n == 0:
                        continue
                    if pi in which_parts:
                        c0 = int(br["pstart"][slot, pi])
                        for g0 in range(0, n, GMAX):
                            gn = min(GMAX, n - g0)
                            t = gat_p.tile([P, gn * elem], BF16,
                                           tag=f"gat{elem}",
                                           name=f"gat{elem}")
                            nc.gpsimd.dma_gather(
                                out_ap=t[:, :gn * elem].rearrange(
                                    "p (k c) -> p k c", k=gn),
                                in_ap=fulls[pi][:],
                                idxs_ap=idx_t[:, (c0 + g0) * 8:
                                              (c0 + g0 + gn) * 8],
                                num_idxs=gn * P,
                                num_idxs_reg=gn * P,
                                elem_size=elem,
                                queue_num=q_rr[0])
                            q_rr[0] = (q_rr[0] + 1) % 4
                            tiles.append((t, joff + g0, gn))
                    joff += n
                return tiles

            def chunk_src(tiles, elem):
                def chunk_ap(j, width):
                    for t, j0, gn in tiles:
                        if j0 <= j < j0 + gn:
                            o = (j - j0) * elem
                            return t[:, o:o + width]
                    raise IndexError(j)
                return chunk_ap

            def emit_gathers(br, idx_t, fulls, slot, elem, tag):
                nparts = len(br["parts"])
                tiles = emit_gathers_parts(br, idx_t, fulls, slot, elem,
                                           set(range(nparts)))
                return chunk_src(tiles, elem)

            def elu_fm(pv, bias_ap, m):
                tmin = elu_p.tile([P, 2 * P], F32, tag="tmin")
                nc.vector.tensor_scalar(
                    out=tmin[:m, :P], in0=pv, scalar1=bias_ap,
                    scalar2=0.0, op0=ALU.add, op1=ALU.min)
                texp = elu_p.tile([P, 2 * P], F32, tag="texp")
                nc.scalar.activation(texp[:m, :P], tmin[:m, :P], AF.Exp)
                trelu = elu_p.tile([P, 2 * P], F32, tag="trelu")
                nc.scalar.activation(trelu[:m, :P], pv, AF.Relu, bias=bias_ap)
                tsum = elu_p.tile([P, 2 * P], F32, tag="tsum")
                nc.vector.tensor_tensor(
                    out=tsum[:m, :P], in0=trelu[:m, :P],
                    in1=texp[:m, :P], op=ALU.add)
                return tsum

            def emit_layer12(pre, br, li, slot, chunk_src, dstl_t,
                             xin_store, xout_store, shard_t):
                o, c = dims[li - 1]
                och = math.ceil(o / P)
                nch = br["nch"]
                starts = br["pstart"][:, 0]
                WrT = w_t[f"{pre}W{li}rT"]
                WnT = w_t[f"{pre}W{li}nT"]
                bias = w_t[f"{pre}B{li}"]
                n_j = int(nch[slot])
                c0 = int(starts[slot])
                oh_t = make_oh(dstl_t, c0, n_j)

                agg_t = psA.tile([P, P], F32, space="PSUM", tag="agg")
                for j in range(n_j):
                    nc.tensor.matmul(
                        out=agg_t[:c, :P],
                        lhsT=chunk_src(j, c),
                        rhs=oh_t[:, j * P:(j + 1) * P],
                        start=(j == 0), stop=(j == n_j - 1))
                aggsb = ev_p.tile([P, P], BF16, tag="aggsb")
                nc.scalar.copy(out=aggsb[:c, :P], in_=agg_t[:c, :P])

                out_t = psB.tile([P, och * P], F32, space="PSUM", tag="out")
                for oc in range(och):
                    o0, o1 = oc * P, min(o, (oc + 1) * P)
                    nc.tensor.matmul(
                        out=out_t[:o1 - o0, oc * P:oc * P + P],
                        lhsT=WrT[:c, o0:o1],
                        rhs=aggsb[:c, :P],
                        start=True, stop=False)
                    nc.tensor.matmul(
                        out=out_t[:o1 - o0, oc * P:oc * P + P],
                        lhsT=WnT[:c, o0:o1],
                        rhs=xin_store[:c, slot * P:slot * P + P],
                        start=False, stop=True)

                for oc in range(och):
                    o0, o1 = oc * P, min(o, (oc + 1) * P)
                    m = o1 - o0
                    pv = out_t[:m, oc * P:oc * P + P]
                    telu = elu_fm(pv, bias[:m, oc:oc + 1], m)
                    nc.scalar.activation(
                        xout_store[:m,
                                   oc * br["slots"] * P + slot * P:
                                   oc * br["slots"] * P + slot * P + P],
                        telu[:m, :P], AF.Identity, bias=neg1_t[:m, 0:1])
                    if li == 1:
                        tps = psT.tile([P, P], F32, space="PSUM", tag="tps")
                        nc.tensor.transpose(
                            out=tps[:, :m], in_=telu[:m, :P],
                            identity=ident_t[:m, :m])
                        stg = st_p.tile([P, P], BF16, tag="stg")
                        nc.scalar.activation(stg[:, :m], tps[:, :m],
                                             AF.Identity, bias=neg1_t[:, 0:1])
                        nc.sync.dma_start(
                            out=shard_t[slot * P:(slot + 1) * P, o0:o1],
                            in_=stg[:, :m])
                if li == 2:
                    emit_y3(pre, br, xout_store, shard_t, slot)

            def emit_y3(pre, br, xin_store, shard_t, slot):
                o, c = dims[2]
                WrT = w_t[f"{pre}W3rT"]
                kch = math.ceil(c / P)
                och = math.ceil(o / P)
                for oc in range(och):
                    o0, o1 = oc * P, min(o, (oc + 1) * P)
                    m = o1 - o0
                    y_t = psA.tile([P, P], F32, space="PSUM", tag="agg")
                    for kc in range(kch):
                        k0, k1 = kc * P, min(c, (kc + 1) * P)
                        nc.tensor.matmul(
                            out=y_t[:m, :P],
                            lhsT=WrT[:k1 - k0, kc * o + o0:kc * o + o1],
                            rhs=xin_store[:k1 - k0,
                                          kc * br["slots"] * P + slot * P:
                                          kc * br["slots"] * P
                                          + slot * P + P],
                            start=(kc == 0), stop=(kc == kch - 1))
                    ysb = elu_p.tile([P, 2 * P], F32, tag="telu")
                    nc.scalar.copy(out=ysb[:m, :P], in_=y_t[:m, :P])
                    tps = psT.tile([P, P], F32, space="PSUM", tag="tps")
                    nc.tensor.transpose(out=tps[:, :m], in_=ysb[:m, :P],
                                        identity=ident_t[:m, :m])
                    stg = st_p.tile([P, P], BF16, tag="stg")
                    nc.scalar.copy(out=stg[:, :m], in_=tps[:, :m])
                    if m < P:
                        nc.vector.memset(stg[:, m:P], 0.0)
                    nc.sync.dma_start(
                        out=shard_t[slot * P:(slot + 1) * P,
                                    oc * P:(oc + 1) * P],
                        in_=stg[:, :P])

            def emit_layer3(pre, br, slot, chunk_src, dstl_t, xin_store,
                            ohB_t, pool_ps):
                o, c = dims[2]
                kch = math.ceil(c / P)
                nch = br["nch"]
                starts = br["pstart"][:, 0]
                WnT = w_t[f"{pre}W3nT"]
                bias_nm = w_t[f"{pre}B3nm"]
                n_j = int(nch[slot])
                c0 = int(starts[slot])
                oh_t = make_oh(dstl_t, c0, n_j)

                ps3 = psB.tile([P, 2 * P], F32, space="PSUM", tag="out")
                for j in range(n_j):
                    nc.tensor.matmul(
                        out=ps3[:, :O3],
                        lhsT=oh_t[:, j * P:(j + 1) * P],
                        rhs=chunk_src(j, O3),
                        start=(j == 0), stop=False)
                for kc in range(kch):
                    k0, k1 = kc * P, min(c, (kc + 1) * P)
                    nc.tensor.matmul(
                        out=ps3[:, :O3],
                        lhsT=xin_store[:k1 - k0,
                                       kc * br["slots"] * P + slot * P:
                                       kc * br["slots"] * P + slot * P + P],
                        rhs=WnT[:k1 - k0, kc * o:kc * o + O3],
                        start=False, stop=False)
                nc.tensor.matmul(
                    out=ps3[:, :O3],
                    lhsT=ones_t[0:1, :P],
                    rhs=bias_nm[0:1, :O3],
                    start=False, stop=True)
                tsum = _elu_pool_nm(ps3, bias_nm, ohB_t, slot, pool_ps)
                if dbg_x4 is not None and pre == "g":
                    x4d = elu_p.tile([P, 2 * P], F32, tag="x4d")
                    nc.vector.tensor_scalar(
                        out=x4d[:, :O3], in0=tsum[:, :O3],
                        scalar1=-1.0, scalar2=None, op0=ALU.add)
                    nc.sync.dma_start(
                        out=dbg_x4[slot * P:(slot + 1) * P, :],
                        in_=x4d[:, :O3])

            def _elu_pool_nm(ps3, bias_nm, ohB_t, slot, pool_ps):
                texp = elu_p.tile([P, 2 * P], F32, tag="texp")
                nc.scalar.activation(texp[:, :O3], ps3[:, :O3], AF.Exp)
                trelu = elu_p.tile([P, 2 * P], F32, tag="trelu")
                nc.scalar.activation(trelu[:, :O3], ps3[:, :O3], AF.Relu)
                tmin = elu_p.tile([P, 2 * P], F32, tag="tmin")
                nc.vector.tensor_scalar(
                    out=tmin[:, :O3], in0=texp[:, :O3],
                    scalar1=1.0, scalar2=None, op0=ALU.min)
                tsum = elu_p.tile([P, 2 * P], F32, tag="tsum")
                nc.vector.tensor_tensor(
                    out=tsum[:, :O3], in0=trelu[:, :O3], in1=tmin[:, :O3],
                    op=ALU.add)
                plps, first, last = pool_ps
                nc.tensor.matmul(
                    out=plps[:B, :O3],
                    lhsT=ohB_t[:, slot * B:(slot + 1) * B],
                    rhs=tsum[:, :O3],
                    start=first, stop=last)
                return tsum

            def e1_chunk_src(e1_in, br, slot):
                n_j = int(br["nch"][slot])
                c0 = int(br["pstart"][slot, 0])
                et = e1_p.tile([P, n_j * NF], BF16, tag="e1")
                nc.sync.dma_start(
                    out=et[:], in_=e1_in[:, c0 * NF:(c0 + n_j) * NF])
                return lambda j, width: et[:, j * NF:j * NF + width]

            g_part_end = {s1: pi for pi, (s0, s1) in enumerate(g["parts"])}
            for slot in range(g["slots"]):
                emit_layer12("g", g, 1, slot,
                             e1_chunk_src(g_e1, g, slot), gdstl_t,
                             xT_store["gA"], xT_store["gB"], gd["x2_shard"])
                if slot + 1 in g_part_end:
                    pi = g_part_end[slot + 1]
                    s0, s1 = g["parts"][pi]
                    ag_part(gd["x2_shard"], gd["x2_full"][pi], s0, s1)
            for slot in range(s["slots"]):
                emit_layer12("s", s, 1, slot,
                             e1_chunk_src(s_e1, s, slot), sdstl_t,
                             xT_store["sA"], xT_store["sB"], sd["x2_shard"])
            ag_part(sd["x2_shard"], sd["x2_full"][0], 0, s["slots"])

            PREF = 6
            y3_parts_fired = set()

            def maybe_fire_g_y3(slot_done):
                if slot_done + 1 in g_part_end:
                    pi = g_part_end[slot_done + 1]
                    s0, s1 = g["parts"][pi]
                    ag_part(gd["y3_shard"], gd["y3_full"][pi], s0, s1)
                    y3_parts_fired.add(pi)

            plg = psPool.tile([P, 2 * P], F32, space="PSUM", tag="plg",
                              name="plg")
            pls = psPool.tile([P, 2 * P], F32, space="PSUM", tag="pls",
                              name="pls")

            nparts_g = len(g["parts"])
            rest_g = set(range(1, nparts_g))

            def g_slot_gathers(pre_map, fulls, slot, elem):
                tiles = pre_map.pop(slot, None)
                if tiles is not None:
                    tiles = tiles + emit_gathers_parts(
                        g, gidx_t, fulls, slot, elem, rest_g)
                    return chunk_src(tiles, elem)
                return chunk_src(emit_gathers_parts(
                    g, gidx_t, fulls, slot, elem,
                    set(range(nparts_g))), elem)

            def emit_s2(ss):
                cs = emit_gathers(s, sidx_t, sd["x2_full"], ss, 2 * NF,
                                  "sx2")
                emit_layer12("s", s, 2, ss, cs, sdstl_t,
                             xT_store["sB"], xT_store["sA"], sd["y3_shard"])
                if ss == s["slots"] - 1:
                    ag_part(sd["y3_shard"], sd["y3_full"][0], 0, s["slots"])

            def emit_s3(ss):
                cs = emit_gathers(s, sidx_t, sd["y3_full"], ss, Y3PAD,
                                  "sy3")
                emit_layer3("s", s, ss, cs, sdstl_t, xT_store["sA"],
                            sohB_t, (pls, ss == 0, ss == s["slots"] - 1))

            s2_at = 12
            s3_at = 30
            s2_done = 0
            s3_done = 0

            pre2 = {}
            for slot in range(min(PREF, g["slots"])):
                pre2[slot] = emit_gathers_parts(
                    g, gidx_t, gd["x2_full"], slot, 2 * NF, {0})
            for slot in range(g["slots"]):
                cs = g_slot_gathers(pre2, gd["x2_full"], slot, 2 * NF)
                emit_layer12("g", g, 2, slot, cs, gdstl_t,
                             xT_store["gB"], xT_store["gA"], gd["y3_shard"])
                if s2_at <= slot < s2_at + s["slots"]:
                    emit_s2(slot - s2_at)
                    s2_done += 1
                if s3_at <= slot < s3_at + s["slots"] and \
                        s2_done == s["slots"]:
                    emit_s3(slot - s3_at)
                    s3_done += 1
                maybe_fire_g_y3(slot)
            for ss in range(s2_done, s["slots"]):
                emit_s2(ss)
                s2_done += 1
            assert len(y3_parts_fired) == nparts_g

            for ss in range(s3_done, s["slots"]):
                emit_s3(ss)
                s3_done += 1
            pre3 = {}
            for slot in range(min(PREF, g["slots"])):
                pre3[slot] = emit_gathers_parts(
                    g, gidx_t, gd["y3_full"], slot, Y3PAD, {0})
            for slot in range(g["slots"]):
                cs = g_slot_gathers(pre3, gd["y3_full"], slot, Y3PAD)
                emit_layer3("g", g, slot, cs, gdstl_t, xT_store["gA"],
                            gohB_t, (plg, slot == 0, slot == g["slots"] - 1))
            nc.scalar.copy(out=poolacc[:B, 0:O3], in_=plg[:B, :O3])
            nc.scalar.copy(out=poolacc[:B, O3:2 * O3], in_=pls[:B, :O3])

            pool_sb = poolacc
            arsb = cp.tile([P, 4 * B], F32, tag="arsb")
            nc.vector.memset(arsb[:], 0.0)
            blocks = [("g", 0, P), ("g", 1, O3 - P), ("s", 0, P),
                      ("s", 1, O3 - P)]
            for bi, (pre, ci, m) in enumerate(blocks):
                base = 0 if pre == "g" else O3
                tps = psT.tile([P, P], F32, space="PSUM", tag="tps")
                nc.tensor.transpose(
                    out=tps[:m, :B],
                    in_=pool_sb[:B, base + ci * P:base + ci * P + m],
                    identity=ident_t[:B, :B])
                nc.scalar.copy(out=arsb[:m, bi * B:(bi + 1) * B],
                               in_=tps[:m, :B])
            nc.sync.dma_start(out=ar_in[:], in_=arsb[:])
            nc.gpsimd.collective_compute(
                "AllReduce", ALU.add,
                replica_groups=[list(range(NCORES))],
                ins=[ar_in.opt()], outs=[ar_out.opt()])
            arres = cp.tile([P, 4 * B], F32, tag="arres")
            nc.sync.dma_start(out=arres[:], in_=ar_out[:])

            zt = cp.tile([P, 5 * B], F32, tag="zt")
            nc.vector.memset(zt[:], 0.0)
            for bi, (pre, ci, m) in enumerate(blocks):
                rec = grec_t if pre == "g" else srec_t
                cor = gcorr_t if pre == "g" else scorr_t
                nc.vector.tensor_tensor(
                    out=zt[:m, bi * B:(bi + 1) * B],
                    in0=arres[:m, bi * B:(bi + 1) * B],
                    in1=rec[:m, :], op=ALU.mult)
                nc.vector.tensor_tensor(
                    out=zt[:m, bi * B:(bi + 1) * B],
                    in0=zt[:m, bi * B:(bi + 1) * B],
                    in1=cor[:m, :], op=ALU.subtract)
            nc.vector.tensor_copy(out=zt[:NF, 4 * B:5 * B], in_=pointT_t[:])

            h1 = cp.tile([P, 5 * B], F32, tag="h1")
            nc.vector.memset(h1[:], 0.0)
            for mchunk in range(5):
                m0, m1 = mchunk * P, min(600, (mchunk + 1) * P)
                hps = psT.tile([P, P], F32, space="PSUM", tag="tps")
                for k in range(5):
                    nc.tensor.matmul(
                        out=hps[:m1 - m0, :B],
                        lhsT=l1w_t[:, k * 600 + m0:k * 600 + m1],
                        rhs=zt[:, k * B:(k + 1) * B],
                        start=(k == 0), stop=(k == 4))
                nc.scalar.activation(
                    h1[:m1 - m0, mchunk * B:(mchunk + 1) * B],
                    hps[:m1 - m0, :B], AF.Relu,
                    bias=l1b_t[:m1 - m0, mchunk:mchunk + 1])
            h2 = cp.tile([P, 2 * B], F32, tag="h2")
            nc.vector.memset(h2[:], 0.0)
            for mchunk in range(2):
                m0 = mchunk * P
                hps = psT.tile([P, P], F32, space="PSUM", tag="tps")
                for k in range(5):
                    nc.tensor.matmul(
                        out=hps[:, :B],
                        lhsT=l2w_t[:, k * 256 + m0:k * 256 + m0 + P],
                        rhs=h1[:, k * B:(k + 1) * B],
                        start=(k == 0), stop=(k == 4))
                nc.scalar.activation(
                    h2[:, mchunk * B:(mchunk + 1) * B], hps[:, :B], AF.Relu,
                    bias=l2b_t[:, mchunk:mchunk + 1])
            ops = psT.tile([P, P], F32, space="PSUM", tag="tps")
            for k in range(2):
                nc.tensor.matmul(
                    out=ops[:NF, :B], lhsT=l3w_t[:, k * NF:(k + 1) * NF],
                    rhs=h2[:, k * B:(k + 1) * B],
                    start=(k == 0), stop=(k == 1))
            o3sb = cp.tile([NF, B], F32, tag="o3sb")
            nc.scalar.activation(o3sb[:], ops[:NF, :B], AF.Identity,
                                 bias=l3b_t[:NF, 0:1])
            tfin = psT.tile([P, P], F32, space="PSUM", tag="tps")
            nc.tensor.transpose(out=tfin[:B, :NF], in_=o3sb[:],
                                identity=ident_t[:NF, :NF])
            osb = cp.tile([B, NF], F32, tag="osb")
            nc.scalar.copy(out=osb[:], in_=tfin[:B, :NF])
            nc.sync.dma_start(out=out_ext[:], in_=osb[:])

            if debug:
                def dump(name, src_t, rows, cols, dtype):
                    d = nc.dram_tensor(name, [rows, cols], dtype,
                                       kind="ExternalOutput")
                    for r0 in range(0, rows, P):
                        r1 = min(rows, r0 + P)
                        bt = st_p.tile([P, cols], dtype, tag="dump")
                        nc.sync.dma_start(out=bt[:r1 - r0, :],
                                          in_=src_t[r0:r1, :])
                        nc.sync.dma_start(out=d[r0:r1, :],
                                          in_=bt[:r1 - r0, :])
                dump("dbg_xg2", gd["x2_shard"], g["S"], 2 * NF, BF16)
                dump("dbg_yg3", gd["y3_shard"], g["S"], Y3PAD, BF16)
                dump("dbg_ar", ar_out, P, 4 * B, F32)
                dump("dbg_arin", ar_in, P, 4 * B, F32)
                dbg_zt = nc.dram_tensor("dbg_zt", [P, 5 * B], F32,
                                        kind="ExternalOutput")
                zt_dump = cp.tile([P, 5 * B], F32, tag="zt_dump")
                nc.vector.tensor_copy(out=zt_dump[:], in_=zt[:])
                nc.sync.dma_start(out=dbg_zt[:], in_=zt_dump[:])

    nc.compile()
    return nc



def kernel(**inputs):
    cfg = CFG_FULL
    inputs = {k: np.asarray(v) for k, v in inputs.items()}
    meta, in_maps = host_prep(inputs, cfg)
    nc = build_program(meta)
    trace = bool(int(os.environ.get("KERNEL_TRACE", "0")))
    if trace:
        import types
        from trn_agent_boot.trn_boot import _ntff_profile_via_ctypes
        hook = _ntff_profile_via_ctypes('/opt/axon/libaxon_pjrt.so')
        mod = types.ModuleType('antenv.axon_hooks')
        mod.get_axon_ntff_profile_hook = lambda: hook
        sys.modules['antenv.axon_hooks'] = mod
    res = run_bass_kernel_spmd(nc, in_maps, list(range(NCORES)), trace=trace)
    if trace and res.exec_time_ns:
        print(f"HW exec time: {res.exec_time_ns} ns")
    return np.asarray(res.results[0]["out"], np.float32)


# revision 13
# speedup vs baseline: 1.0292x; 1.0292x over previous
import os
import sys
import math

sys.path.insert(0, "/opt/trn_rl_repo")

import numpy as np
import ml_dtypes

import concourse.bass as bass
import concourse.mybir as mybir
import concourse.tile as tile
from concourse import bacc
from concourse import library_config
from concourse.bass_utils import run_bass_kernel_spmd

P = 128
NCORES = 8
B = 8
BF16 = mybir.dt.bfloat16
F32 = mybir.dt.float32
I16 = mybir.dt.int16
AF = mybir.ActivationFunctionType
ALU = mybir.AluOpType

CFG_FULL = dict(N_G=50000, E_G=800000, N_S=10000, E_S=160000, NF=64)

Y3PAD = 256



def _assign_nodes(dst, n_nodes):
    import heapq
    deg = np.bincount(dst, minlength=n_nodes)
    slots = math.ceil(n_nodes / (NCORES * P))
    nbins = NCORES * slots
    order = np.argsort(-deg, kind="stable")
    heap = [(0, 0, b) for b in range(nbins)]
    heapq.heapify(heap)
    bin_of = np.empty(n_nodes, np.int64)
    bin_fill = np.zeros(nbins, np.int64)
    pos_of = np.empty(n_nodes, np.int64)
    for n in order:
        while True:
            load, cnt, b = heapq.heappop(heap)
            if bin_fill[b] < P:
                break
        bin_of[n] = b
        pos_of[n] = bin_fill[b]
        bin_fill[b] += 1
        if bin_fill[b] < P:
            heapq.heappush(heap, (load + int(deg[n]), cnt + 1, b))
    bin_load = np.zeros(nbins, np.int64)
    np.add.at(bin_load, bin_of[dst], 1)
    core_of_bin = np.arange(nbins) % NCORES
    slot_of_bin = np.empty(nbins, np.int64)
    for c in range(NCORES):
        bins_c = np.where(core_of_bin == c)[0]
        order_c = bins_c[np.argsort(-bin_load[bins_c], kind="stable")]
        slot_of_bin[order_c] = np.arange(slots)
    core_of = core_of_bin[bin_of]
    slot_of = slot_of_bin[bin_of]
    perm = core_of * (slots * P) + slot_of * P + pos_of
    return perm, core_of, slot_of, pos_of, slots


def _build_edges(src, dst, perm, core_of, slot_of, pos_of, slots, parts):
    S = slots * P
    nparts = len(parts)
    ecore = core_of[dst]
    eslot = slot_of[dst]
    psrc = perm[src]
    pc = psrc // S
    pr = psrc % S
    ps_slot = pr // P
    part_of = np.zeros(len(src), np.int64)
    rowid = np.empty(len(src), np.int64)
    for pi, (s0, s1) in enumerate(parts):
        m = (ps_slot >= s0) & (ps_slot < s1)
        part_of[m] = pi
        Sp = (s1 - s0) * P
        rowid[m] = pc[m] * Sp + (pr[m] - s0 * P)
        assert Sp * NCORES <= 32768
    cnt = np.zeros((NCORES, slots, nparts), np.int64)
    np.add.at(cnt, (ecore, eslot, part_of), 1)
    nch_p = np.ceil(cnt.max(axis=0) / P).astype(np.int64)
    nch_p[:, 0] = np.maximum(1, nch_p[:, 0])
    nch = nch_p.sum(axis=1)
    tc = int(nch.sum())
    pstart = np.zeros((slots, nparts), np.int64)
    col = 0
    for s in range(slots):
        for pi in range(nparts):
            pstart[s, pi] = col
            col += nch_p[s, pi]
    src_arr = np.zeros((NCORES, P, tc), np.int32)
    dstl_arr = np.full((NCORES, P, tc), 255.0, np.float32)
    idx16 = np.zeros((NCORES, 16, tc * 8), np.int16)
    order = np.lexsort((part_of, eslot, ecore))
    ec, esl, ep = ecore[order], eslot[order], part_of[order]
    psrc_o = psrc[order].astype(np.int32)
    rid_o = rowid[order].astype(np.int16)
    pdst = pos_of[dst][order].astype(np.float32)
    key = (ec * slots + esl) * nparts + ep
    bounds = np.searchsorted(key, np.arange(NCORES * slots * nparts + 1))
    for c in range(NCORES):
        for s in range(slots):
            for pi in range(nparts):
                k = (c * slots + s) * nparts + pi
                a, b = bounds[k], bounds[k + 1]
                n = b - a
                ncol = int(nch_p[s, pi])
                if ncol == 0:
                    assert n == 0
                    continue
                c0 = int(pstart[s, pi])
                buf_s = np.zeros(P * ncol, np.int32)
                buf_d = np.full(P * ncol, 255.0, np.float32)
                buf_i = np.zeros(P * ncol, np.int16)
                buf_s[:n] = psrc_o[a:b]
                buf_d[:n] = pdst[a:b]
                buf_i[:n] = rid_o[a:b]
                src_arr[c, :, c0:c0 + ncol] = buf_s.reshape(ncol, P).T
                dstl_arr[c, :, c0:c0 + ncol] = buf_d.reshape(ncol, P).T
                idx16[c, :, c0 * 8:(c0 + ncol) * 8] = \
                    buf_i.reshape(ncol * 8, 16).T
    return src_arr, dstl_arr, idx16, nch_p, pstart, nch.astype(int), tc


def _prep_branch(x, edge_index, batch, n_nodes):
    src = np.asarray(edge_index[0], np.int64)
    dst = np.asarray(edge_index[1], np.int64)
    perm, core_of, slot_of, pos_of, slots = _assign_nodes(dst, n_nodes)
    max_slots_per_part = 32768 // (NCORES * P)
    nparts = math.ceil(slots / max_slots_per_part)
    bnds = np.linspace(0, slots, nparts + 1).astype(int)
    parts = [(int(bnds[i]), int(bnds[i + 1])) for i in range(nparts)]
    src_arr, dstl_arr, idx16, nch_p, pstart, nch, tc = _build_edges(
        src, dst, perm, core_of, slot_of, pos_of, slots, parts)
    npad = NCORES * slots * P
    S = slots * P
    nf = x.shape[1]
    x_full = np.zeros((npad, nf), np.float32)
    x_full[perm] = np.asarray(x, np.float32)
    xT = np.stack([x_full[c * S:(c + 1) * S].T.copy() for c in range(NCORES)])
    bf = ml_dtypes.bfloat16
    xf16 = x_full.astype(bf)
    e1 = np.stack([xf16[src_arr[c]].reshape(P, tc * nf) for c in range(NCORES)])
    ohB = np.zeros((NCORES, P, slots * B), np.float32)
    bvec = np.asarray(batch, np.int64)
    pid = perm
    c_all, r_all = np.divmod(pid, S)
    s_all, p_all = np.divmod(r_all, P)
    for n in range(n_nodes):
        ohB[c_all[n], p_all[n], s_all[n] * B + int(bvec[n])] = 1.0
    cntb = np.bincount(bvec, minlength=B).astype(np.float32)
    recip = 1.0 / np.maximum(cntb, 1.0)
    corr = (cntb > 0).astype(np.float32)
    return dict(perm=perm, slots=slots, S=S, npad=npad, nch=nch, tc=tc,
                parts=parts, nch_p=nch_p, pstart=pstart,
                src_arr=src_arr, dstl_arr=dstl_arr, idx16=idx16,
                x_full=x_full, xT=xT, e1=e1, ohB=ohB, recip=recip, corr=corr)


def _pack_wt(w):
    return np.ascontiguousarray(np.asarray(w, np.float32).T)


def _pack_bias(bvec, nchunks):
    out = np.zeros((P, nchunks), np.float32)
    b = np.asarray(bvec, np.float32)
    for j in range(nchunks):
        seg = b[j * P:(j + 1) * P]
        out[:len(seg), j] = seg
    return out


def host_prep(inputs, cfg):
    g = _prep_branch(inputs["graph_x"], inputs["graph_edge_index"],
                     inputs["graph_batch"], cfg["N_G"])
    s = _prep_branch(inputs["subgraph_x"], inputs["subgraph_edge_index"],
                     inputs["subgraph_batch"], cfg["N_S"])
    NF = cfg["NF"]
    meta = dict(g=g, s=s, NF=NF)

    bf = ml_dtypes.bfloat16
    common = {}
    common["iota"] = np.broadcast_to(
        np.arange(P, dtype=np.float32), (P, P)).copy()
    common["ident"] = np.eye(P, dtype=np.float32)

    dims = [(2 * NF, NF), (4 * NF, 2 * NF), (3 * NF, 4 * NF)]
    meta["dims"] = dims
    for pre in ("g", "s"):
        for li, (o, c) in enumerate(dims, start=1):
            common[f"{pre}W{li}rT"] = _pack_wt(inputs[f"{pre}W{li}r"]).astype(bf)
            common[f"{pre}W{li}nT"] = _pack_wt(inputs[f"{pre}W{li}n"]).astype(bf)
            common[f"{pre}B{li}"] = _pack_bias(inputs[f"{pre}B{li}"],
                                               math.ceil(o / P))
        b3 = np.asarray(inputs[f"{pre}B3"], np.float32)
        common[f"{pre}B3nm"] = np.broadcast_to(b3, (P, 3 * NF)).copy()
    O3 = 3 * NF
    zmap = np.full(5 * P, -1, np.int64)
    zmap[0:P] = np.arange(0, P)
    zmap[P:P + (O3 - P)] = np.arange(P, O3)
    zmap[2 * P:3 * P] = O3 + np.arange(0, P)
    zmap[3 * P:3 * P + (O3 - P)] = O3 + np.arange(P, O3)
    zmap[4 * P:4 * P + NF] = 2 * O3 + np.arange(NF)
    l1W = np.asarray(inputs["l1W"], np.float32)
    l1WT = np.zeros((5 * P, 600), np.float32)
    valid = zmap >= 0
    l1WT[valid] = l1W[:, zmap[valid]].T
    H1, H2 = 600, 256
    M1 = math.ceil(H1 / P)
    l2W = np.asarray(inputs["l2W"], np.float32)
    l2WT = np.zeros((M1 * P, H2), np.float32)
    l2WT[:H1] = l2W.T
    l3W = np.asarray(inputs["l3W"], np.float32)
    l3WT = np.ascontiguousarray(l3W.T)

    def pack_k(wt, kchunks, width):
        out = np.zeros((P, kchunks * width), np.float32)
        for k in range(kchunks):
            seg = wt[k * P:(k + 1) * P]
            out[:seg.shape[0], k * width:k * width + width] = seg
        return out

    common["l1WT"] = pack_k(l1WT, 5, 600)
    common["l2WT"] = pack_k(l2WT, M1, H2)
    common["l3WT"] = pack_k(l3WT, 2, NF)
    common["l1b"] = _pack_bias(inputs["l1b"], M1)
    common["l2b"] = _pack_bias(inputs["l2b"], 2)
    common["l3b"] = _pack_bias(inputs["l3b"], 1)
    common["pointT"] = np.ascontiguousarray(
        np.asarray(inputs["point"], np.float32).T)

    in_maps = []
    for c in range(NCORES):
        m = dict(common)
        m["g_dstl"] = g["dstl_arr"][c]
        m["s_dstl"] = s["dstl_arr"][c]
        m["g_idx"] = np.tile(g["idx16"][c], (8, 1))
        m["s_idx"] = np.tile(s["idx16"][c], (8, 1))
        m["g_e1"] = g["e1"][c]
        m["s_e1"] = s["e1"][c]
        m["xg1T"] = g["xT"][c].astype(bf)
        m["xs1T"] = s["xT"][c].astype(bf)
        m["g_ohB"] = g["ohB"][c]
        m["s_ohB"] = s["ohB"][c]
        m["g_recip"] = np.broadcast_to(g["recip"], (P, B)).astype(
            np.float32).copy()
        m["s_recip"] = np.broadcast_to(s["recip"], (P, B)).astype(
            np.float32).copy()
        m["g_corr"] = np.broadcast_to(g["corr"], (P, B)).astype(
            np.float32).copy()
        m["s_corr"] = np.broadcast_to(s["corr"], (P, B)).astype(
            np.float32).copy()
        in_maps.append(m)
    return meta, in_maps



def _ap3(t_ap, mid_count):
    return bass.AP(t_ap.tensor, t_ap.offset,
                   [list(t_ap.ap[0]), [0, mid_count], list(t_ap.ap[1])])


def build_program(meta, debug=False):
    NF = meta["NF"]
    dims = meta["dims"]
    g, s = meta["g"], meta["s"]
    O3 = 3 * NF

    nc = bacc.Bacc(None, target_bir_lowering=False, debug=False,
                   num_swdge_queues=4)

    def din(name, shape, dtype):
        return nc.dram_tensor(name, list(shape), dtype, kind="ExternalInput")

    xg1T = din("xg1T", [NF, g["S"]], BF16)
    xs1T = din("xs1T", [NF, s["S"]], BF16)
    g_dstl = din("g_dstl", [P, g["tc"]], F32)
    s_dstl = din("s_dstl", [P, s["tc"]], F32)
    g_idx = din("g_idx", [P, g["tc"] * 8], I16)
    s_idx = din("s_idx", [P, s["tc"] * 8], I16)
    g_e1 = din("g_e1", [P, g["tc"] * NF], BF16)
    s_e1 = din("s_e1", [P, s["tc"] * NF], BF16)
    iota_in = din("iota", [P, P], F32)
    ident_in = din("ident", [P, P], F32)
    wts = {}
    for pre in ("g", "s"):
        for li, (o, c) in enumerate(dims, start=1):
            wts[f"{pre}W{li}rT"] = din(f"{pre}W{li}rT", [c, o], BF16)
            wts[f"{pre}W{li}nT"] = din(f"{pre}W{li}nT", [c, o], BF16)
            wts[f"{pre}B{li}"] = din(f"{pre}B{li}", [P, math.ceil(o / P)], F32)
        wts[f"{pre}B3nm"] = din(f"{pre}B3nm", [P, O3], F32)
    l1WT = din("l1WT", [P, 5 * 600], F32)
    l2WT = din("l2WT", [P, 5 * 256], F32)
    l3WT = din("l3WT", [P, 2 * NF], F32)
    l1b = din("l1b", [P, 5], F32)
    l2b = din("l2b", [P, 2], F32)
    l3b = din("l3b", [P, 1], F32)
    pointT = din("pointT", [NF, B], F32)
    g_ohB = din("g_ohB", [P, g["slots"] * B], F32)
    s_ohB = din("s_ohB", [P, s["slots"] * B], F32)
    g_recip = din("g_recip", [P, B], F32)
    s_recip = din("s_recip", [P, B], F32)
    g_corr = din("g_corr", [P, B], F32)
    s_corr = din("s_corr", [P, B], F32)

    out_ext = nc.dram_tensor("out", [B, NF], F32, kind="ExternalOutput")

    with tile.TileContext(nc) as tc:
        with tc.tile_pool(name="const", bufs=1) as cp, \
             tc.tile_pool(name="gat", bufs=10) as gat_p, \
             tc.tile_pool(name="e1p", bufs=3) as e1_p, \
             tc.tile_pool(name="oh", bufs=3) as oh_p, \
             tc.tile_pool(name="evac", bufs=3) as ev_p, \
             tc.tile_pool(name="elu", bufs=2) as elu_p, \
             tc.tile_pool(name="stage", bufs=3) as st_p, \
             tc.tile_pool(name="psA", bufs=2, space="PSUM") as psA, \
             tc.tile_pool(name="psB", bufs=3, space="PSUM") as psB, \
             tc.tile_pool(name="psT", bufs=1, space="PSUM") as psT, \
             tc.tile_pool(name="psPool", bufs=1, space="PSUM") as psPool, \
             tc.tile_pool(name="dram", bufs=1, space="DRAM") as dram:

            nc.gpsimd.load_library(library_config.mlp)

            def load_const(name, src_t, shape, dtype):
                t = cp.tile(list(shape), dtype, tag=name)
                nc.sync.dma_start(out=t[:], in_=src_t[:])
                return t

            gdstl_t = load_const("g_dstl", g_dstl, [P, g["tc"]], F32)
            sdstl_t = load_const("s_dstl", s_dstl, [P, s["tc"]], F32)
            gidx_t = load_const("g_idx", g_idx, [P, g["tc"] * 8], I16)
            sidx_t = load_const("s_idx", s_idx, [P, s["tc"] * 8], I16)
            iota_t = load_const("iota", iota_in, [P, P], F32)
            ident_t = load_const("ident", ident_in, [P, P], F32)
            w_t = {}
            for pre in ("g", "s"):
                for li, (o, c) in enumerate(dims, start=1):
                    for rn in ("r", "n"):
                        nm = f"{pre}W{li}{rn}T"
                        kch = math.ceil(c / P)
                        t = cp.tile([P, kch * o], BF16, tag=nm)
                        src_w = wts[nm]
                        if c < P:
                            nc.sync.dma_start(out=t[:c, :o], in_=src_w[:])
                        else:
                            nc.sync.dma_start(
                                out=t[:].rearrange("p (k o) -> p k o", k=kch),
                                in_=src_w[:].rearrange("(k p) o -> p k o",
                                                       p=P))
                        w_t[nm] = t
                    nm = f"{pre}B{li}"
                    w_t[nm] = load_const(nm, wts[nm],
                                         [P, math.ceil(o / P)], F32)
                nm = f"{pre}B3nm"
                w_t[nm] = load_const(nm, wts[nm], [P, O3], F32)
            l1w_t = load_const("l1WT", l1WT, [P, 5 * 600], F32)
            l2w_t = load_const("l2WT", l2WT, [P, 5 * 256], F32)
            l3w_t = load_const("l3WT", l3WT, [P, 2 * NF], F32)
            l1b_t = load_const("l1b", l1b, [P, 5], F32)
            l2b_t = load_const("l2b", l2b, [P, 2], F32)
            l3b_t = load_const("l3b", l3b, [P, 1], F32)
            pointT_t = load_const("pointT", pointT, [NF, B], F32)
            gohB_t = load_const("g_ohB", g_ohB, [P, g["slots"] * B], F32)
            sohB_t = load_const("s_ohB", s_ohB, [P, s["slots"] * B], F32)
            grec_t = load_const("g_recip", g_recip, [P, B], F32)
            srec_t = load_const("s_recip", s_recip, [P, B], F32)
            gcorr_t = load_const("g_corr", g_corr, [P, B], F32)
            scorr_t = load_const("s_corr", s_corr, [P, B], F32)
            neg1_t = cp.tile([P, 1], F32, tag="neg1", name="neg1")
            nc.vector.memset(neg1_t[:], -1.0)
            ones_t = cp.tile([P, P], F32, tag="ones", name="ones")
            nc.vector.memset(ones_t[:], 1.0)

            xT_store = {}
            for pre, br in (("g", g), ("s", s)):
                for ab in "AB":
                    xT_store[pre + ab] = cp.tile(
                        [P, 2 * br["slots"] * P], BF16,
                        tag=f"xT{pre}{ab}", name=f"xT{pre}{ab}")
            nc.sync.dma_start(out=xT_store["gA"][:NF, :g["S"]], in_=xg1T[:])
            nc.sync.dma_start(out=xT_store["sA"][:NF, :s["S"]], in_=xs1T[:])

            def branch_dram(pre, br):
                d = {}
                d["x2_shard"] = dram.tile([br["S"], 2 * NF], BF16,
                                          tag=f"x2sh_{pre}",
                                          name=f"x2sh_{pre}")
                d["y3_shard"] = dram.tile([br["S"], Y3PAD], BF16,
                                          tag=f"y3sh_{pre}",
                                          name=f"y3sh_{pre}")
                d["x2_full"] = []
                d["y3_full"] = []
                for pi, (s0, s1) in enumerate(br["parts"]):
                    rows = (s1 - s0) * P * NCORES
                    d["x2_full"].append(dram.tile(
                        [rows, 2 * NF], BF16, tag=f"x2f_{pre}{pi}",
                        name=f"x2f_{pre}{pi}", addr_space="Shared"))
                    d["y3_full"].append(dram.tile(
                        [rows, Y3PAD], BF16, tag=f"y3f_{pre}{pi}",
                        name=f"y3f_{pre}{pi}", addr_space="Shared"))
                return d

            gd = branch_dram("g", g)
            sd = branch_dram("s", s)
            ar_in = dram.tile([P, 4 * B], F32, tag="ar_in")
            ar_out = dram.tile([P, 4 * B], F32, tag="ar_out")

            poolacc = cp.tile([P, 2 * O3], F32, tag="poolacc")
            nc.vector.memset(poolacc[:], 0.0)

            dbg_x4 = None
            if debug:
                dbg_x4 = nc.dram_tensor("dbg_x4", [g["S"], O3], F32,
                                        kind="ExternalOutput")

            def ag_part(shard_t, full_t, s0, s1):
                nc.gpsimd.collective_compute(
                    "AllGather", ALU.bypass,
                    replica_groups=[list(range(NCORES))],
                    ins=[shard_t[s0 * P:s1 * P, :].opt()],
                    outs=[full_t[:].opt()])

            def make_oh(dstl_t, c0, n_j):
                oh_t = oh_p.tile([P, n_j * P], BF16, tag="oh")
                d_ap = dstl_t[:, c0:c0 + n_j].to_broadcast([P, n_j, P])
                i_ap = _ap3(iota_t[:], n_j)
                nc.vector.tensor_tensor(
                    out=oh_t[:].rearrange("p (k q) -> p k q", k=n_j),
                    in0=d_ap, in1=i_ap, op=ALU.is_equal)
                return oh_t

            GMAX = 8
            q_rr = [0]

            def emit_gathers_parts(br, idx_t, fulls, slot, elem,
                                   which_parts):
                tiles = []
                joff = 0
                for pi in range(len(br["parts"])):
                    n = int(br["nch_p"][slot, pi])
                    if n == 0:
                        continue
                    if pi in which_parts:
                        c0 = int(br["pstart"][slot, pi])
                        for g0 in range(0, n, GMAX):
                            gn = min(GMAX, n - g0)
                            t = gat_p.tile([P, gn * elem], BF16,
                                           tag=f"gat{elem}",
                                           name=f"gat{elem}")
                            nc.gpsimd.dma_gather(
                                out_ap=t[:, :gn * elem].rearrange(
                                    "p (k c) -> p k c", k=gn),
                                in_ap=fulls[pi][:],
                                idxs_ap=idx_t[:, (c0 + g0) * 8:
                                              (c0 + g0 + gn) * 8],
                                num_idxs=gn * P,
                                num_idxs_reg=gn * P,
                                elem_size=elem,
                                queue_num=q_rr[0])
                            q_rr[0] = (q_rr[0] + 1) % 4
                            tiles.append((t, joff + g0, gn))
                    joff += n
                return tiles

            def chunk_src(tiles, elem):
                def chunk_ap(j, width):
                    for t, j0, gn in tiles:
                        if j0 <= j < j0 + gn:
                            o = (j - j0) * elem
                            return t[:, o:o + width]
                    raise IndexError(j)
                return chunk_ap

            def emit_gathers(br, idx_t, fulls, slot, elem, tag):
                nparts = len(br["parts"])
                tiles = emit_gathers_parts(br, idx_t, fulls, slot, elem,
                                           set(range(nparts)))
                return chunk_src(tiles, elem)

            def elu_fm(pv, bias_ap, m):
                tmin = elu_p.tile([P, 2 * P], F32, tag="tmin")
                nc.vector.tensor_scalar(
                    out=tmin[:m, :P], in0=pv, scalar1=bias_ap,
                    scalar2=0.0, op0=ALU.add, op1=ALU.min)
                texp = elu_p.tile([P, 2 * P], F32, tag="texp")
                nc.scalar.activation(texp[:m, :P], tmin[:m, :P], AF.Exp)
                trelu = elu_p.tile([P, 2 * P], F32, tag="trelu")
                nc.scalar.activation(trelu[:m, :P], pv, AF.Relu, bias=bias_ap)
                tsum = elu_p.tile([P, 2 * P], F32, tag="tsum")
                nc.vector.tensor_tensor(
                    out=tsum[:m, :P], in0=trelu[:m, :P],
                    in1=texp[:m, :P], op=ALU.add)
                return tsum

            def emit_layer12(pre, br, li, slot, chunk_src, dstl_t,
                             xin_store, xout_store, shard_t):
                o, c = dims[li - 1]
                och = math.ceil(o / P)
                nch = br["nch"]
                starts = br["pstart"][:, 0]
                WrT = w_t[f"{pre}W{li}rT"]
                WnT = w_t[f"{pre}W{li}nT"]
                bias = w_t[f"{pre}B{li}"]
                n_j = int(nch[slot])
                c0 = int(starts[slot])
                oh_t = make_oh(dstl_t, c0, n_j)

                agg_t = psA.tile([P, P], F32, space="PSUM", tag="agg")
                for j in range(n_j):
                    nc.tensor.matmul(
                        out=agg_t[:c, :P],
                        lhsT=chunk_src(j, c),
                        rhs=oh_t[:, j * P:(j + 1) * P],
                        start=(j == 0), stop=(j == n_j - 1))
                aggsb = ev_p.tile([P, P], BF16, tag="aggsb")
                nc.scalar.copy(out=aggsb[:c, :P], in_=agg_t[:c, :P])

                out_t = psB.tile([P, och * P], F32, space="PSUM", tag="out")
                for oc in range(och):
                    o0, o1 = oc * P, min(o, (oc + 1) * P)
                    nc.tensor.matmul(
                        out=out_t[:o1 - o0, oc * P:oc * P + P],
                        lhsT=WrT[:c, o0:o1],
                        rhs=aggsb[:c, :P],
                        start=True, stop=False)
                    nc.tensor.matmul(
                        out=out_t[:o1 - o0, oc * P:oc * P + P],
                        lhsT=WnT[:c, o0:o1],
                        rhs=xin_store[:c, slot * P:slot * P + P],
                        start=False, stop=True)

                for oc in range(och):
                    o0, o1 = oc * P, min(o, (oc + 1) * P)
                    m = o1 - o0
                    pv = out_t[:m, oc * P:oc * P + P]
                    telu = elu_fm(pv, bias[:m, oc:oc + 1], m)
                    nc.scalar.activation(
                        xout_store[:m,
                                   oc * br["slots"] * P + slot * P:
                                   oc * br["slots"] * P + slot * P + P],
                        telu[:m, :P], AF.Identity, bias=neg1_t[:m, 0:1])
                    if li == 1:
                        tps = psT.tile([P, P], F32, space="PSUM", tag="tps")
                        nc.tensor.transpose(
                            out=tps[:, :m], in_=telu[:m, :P],
                            identity=ident_t[:m, :m])
                        stg = st_p.tile([P, P], BF16, tag="stg")
                        nc.scalar.activation(stg[:, :m], tps[:, :m],
                                             AF.Identity, bias=neg1_t[:, 0:1])
                        nc.sync.dma_start(
                            out=shard_t[slot * P:(slot + 1) * P, o0:o1],
                            in_=stg[:, :m])
                if li == 2:
                    emit_y3(pre, br, xout_store, shard_t, slot)

            def emit_y3(pre, br, xin_store, shard_t, slot):
                o, c = dims[2]
                WrT = w_t[f"{pre}W3rT"]
                kch = math.ceil(c / P)
                och = math.ceil(o / P)
                for oc in range(och):
                    o0, o1 = oc * P, min(o, (oc + 1) * P)
                    m = o1 - o0
                    y_t = psA.tile([P, P], F32, space="PSUM", tag="agg")
                    for kc in range(kch):
                        k0, k1 = kc * P, min(c, (kc + 1) * P)
                        nc.tensor.matmul(
                            out=y_t[:m, :P],
                            lhsT=WrT[:k1 - k0, kc * o + o0:kc * o + o1],
                            rhs=xin_store[:k1 - k0,
                                          kc * br["slots"] * P + slot * P:
                                          kc * br["slots"] * P
                                          + slot * P + P],
                            start=(kc == 0), stop=(kc == kch - 1))
                    ysb = elu_p.tile([P, 2 * P], F32, tag="telu")
                    nc.scalar.copy(out=ysb[:m, :P], in_=y_t[:m, :P])
                    tps = psT.tile([P, P], F32, space="PSUM", tag="tps")
                    nc.tensor.transpose(out=tps[:, :m], in_=ysb[:m, :P],
                                        identity=ident_t[:m, :m])
                    stg = st_p.tile([P, P], BF16, tag="stg")
                    nc.scalar.copy(out=stg[:, :m], in_=tps[:, :m])
                    if m < P:
                        nc.vector.memset(stg[:, m:P], 0.0)
                    nc.sync.dma_start(
                        out=shard_t[slot * P:(slot + 1) * P,
                                    oc * P:(oc + 1) * P],
                        in_=stg[:, :P])

            def emit_layer3(pre, br, slot, chunk_src, dstl_t, xin_store,
                            ohB_t, pool_ps):
                o, c = dims[2]
                kch = math.ceil(c / P)
                nch = br["nch"]
                starts = br["pstart"][:, 0]
                WnT = w_t[f"{pre}W3nT"]
                bias_nm = w_t[f"{pre}B3nm"]
                n_j = int(nch[slot])
                c0 = int(starts[slot])
                oh_t = make_oh(dstl_t, c0, n_j)

                ps3 = psB.tile([P, 2 * P], F32, space="PSUM", tag="out")
                for j in range(n_j):
                    nc.tensor.matmul(
                        out=ps3[:, :O3],
                        lhsT=oh_t[:, j * P:(j + 1) * P],
                        rhs=chunk_src(j, O3),
                        start=(j == 0), stop=False)
                for kc in range(kch):
                    k0, k1 = kc * P, min(c, (kc + 1) * P)
                    nc.tensor.matmul(
                        out=ps3[:, :O3],
                        lhsT=xin_store[:k1 - k0,
                                       kc * br["slots"] * P + slot * P:
                                       kc * br["slots"] * P + slot * P + P],
                        rhs=WnT[:k1 - k0, kc * o:kc * o + O3],
                        start=False, stop=False)
                nc.tensor.matmul(
                    out=ps3[:, :O3],
                    lhsT=ones_t[0:1, :P],
                    rhs=bias_nm[0:1, :O3],
                    start=False, stop=True)
                tsum = _elu_pool_nm(ps3, bias_nm, ohB_t, slot, pool_ps)
                if dbg_x4 is not None and pre == "g":
                    x4d = elu_p.tile([P, 2 * P], F32, tag="x4d")
                    nc.vector.tensor_scalar(
                        out=x4d[:, :O3], in0=tsum[:, :O3],
                        scalar1=-1.0, scalar2=None, op0=ALU.add)
                    nc.sync.dma_start(
                        out=dbg_x4[slot * P:(slot + 1) * P, :],
                        in_=x4d[:, :O3])

            def _elu_pool_nm(ps3, bias_nm, ohB_t, slot, pool_ps):
                texp = elu_p.tile([P, 2 * P], F32, tag="texp")
                nc.scalar.activation(texp[:, :O3], ps3[:, :O3], AF.Exp)
                trelu = elu_p.tile([P, 2 * P], F32, tag="trelu")
                nc.scalar.activation(trelu[:, :O3], ps3[:, :O3], AF.Relu)
                tmin = elu_p.tile([P, 2 * P], F32, tag="tmin")
                nc.vector.tensor_scalar(
                    out=tmin[:, :O3], in0=texp[:, :O3],
                    scalar1=1.0, scalar2=None, op0=ALU.min)
                tsum = elu_p.tile([P, 2 * P], F32, tag="tsum")
                nc.vector.tensor_tensor(
                    out=tsum[:, :O3], in0=trelu[:, :O3], in1=tmin[:, :O3],
                    op=ALU.add)
                plps, first, last = pool_ps
                nc.tensor.matmul(
                    out=plps[:B, :O3],
                    lhsT=ohB_t[:, slot * B:(slot + 1) * B],
                    rhs=tsum[:, :O3],
                    start=first, stop=last)
                return tsum

            def e1_chunk_src(e1_in, br, slot):
                n_j = int(br["nch"][slot])
                c0 = int(br["pstart"][slot, 0])
                et = e1_p.tile([P, n_j * NF], BF16, tag="e1")
                nc.sync.dma_start(
                    out=et[:], in_=e1_in[:, c0 * NF:(c0 + n_j) * NF])
                return lambda j, width: et[:, j * NF:j * NF + width]

            g_part_end = {s1: pi for pi, (s0, s1) in enumerate(g["parts"])}
            for slot in range(g["slots"]):
                emit_layer12("g", g, 1, slot,
                             e1_chunk_src(g_e1, g, slot), gdstl_t,
                             xT_store["gA"], xT_store["gB"], gd["x2_shard"])
                if slot + 1 in g_part_end:
                    pi = g_part_end[slot + 1]
                    s0, s1 = g["parts"][pi]
                    ag_part(gd["x2_shard"], gd["x2_full"][pi], s0, s1)
            for slot in range(s["slots"]):
                emit_layer12("s", s, 1, slot,
                             e1_chunk_src(s_e1, s, slot), sdstl_t,
                             xT_store["sA"], xT_store["sB"], sd["x2_shard"])
            ag_part(sd["x2_shard"], sd["x2_full"][0], 0, s["slots"])

            PREF = 6
            y3_parts_fired = set()

            def maybe_fire_g_y3(slot_done):
                if slot_done + 1 in g_part_end:
                    pi = g_part_end[slot_done + 1]
                    s0, s1 = g["parts"][pi]
                    ag_part(gd["y3_shard"], gd["y3_full"][pi], s0, s1)
                    y3_parts_fired.add(pi)

            plg = psPool.tile([P, 2 * P], F32, space="PSUM", tag="plg",
                              name="plg")
            pls = psPool.tile([P, 2 * P], F32, space="PSUM", tag="pls",
                              name="pls")

            nparts_g = len(g["parts"])
            rest_g = set(range(1, nparts_g))

            def g_slot_gathers(pre_map, fulls, slot, elem):
                tiles = pre_map.pop(slot, None)
                if tiles is not None:
                    tiles = tiles + emit_gathers_parts(
                        g, gidx_t, fulls, slot, elem, rest_g)
                    return chunk_src(tiles, elem)
                return chunk_src(emit_gathers_parts(
                    g, gidx_t, fulls, slot, elem,
                    set(range(nparts_g))), elem)

            def emit_s2(ss):
                cs = emit_gathers(s, sidx_t, sd["x2_full"], ss, 2 * NF,
                                  "sx2")
                emit_layer12("s", s, 2, ss, cs, sdstl_t,
                             xT_store["sB"], xT_store["sA"], sd["y3_shard"])
                if ss == s["slots"] - 1:
                    ag_part(sd["y3_shard"], sd["y3_full"][0], 0, s["slots"])

            def emit_s3(ss):
                cs = emit_gathers(s, sidx_t, sd["y3_full"], ss, Y3PAD,
                                  "sy3")
                emit_layer3("s", s, ss, cs, sdstl_t, xT_store["sA"],
                            sohB_t, (pls, ss == 0, ss == s["slots"] - 1))

            s2_at = 12
            s3_at = 30
            s2_done = 0
            s3_done = 0

            pre2 = {}
            for slot in range(min(PREF, g["slots"])):
                pre2[slot] = emit_gathers_parts(
                    g, gidx_t, gd["x2_full"], slot, 2 * NF, {0})
            for slot in range(g["slots"]):
                cs = g_slot_gathers(pre2, gd["x2_full"], slot, 2 * NF)
                emit_layer12("g", g, 2, slot, cs, gdstl_t,
                             xT_store["gB"], xT_store["gA"], gd["y3_shard"])
                if s2_at <= slot < s2_at + s["slots"]:
                    emit_s2(slot - s2_at)
                    s2_done += 1
                if s3_at <= slot < s3_at + min(5, s["slots"]) and \
                        s2_done == s["slots"]:
                    emit_s3(slot - s3_at)
                    s3_done += 1
                maybe_fire_g_y3(slot)
            for ss in range(s2_done, s["slots"]):
                emit_s2(ss)
                s2_done += 1
            assert len(y3_parts_fired) == nparts_g

            for ss in range(s3_done, s["slots"]):
                emit_s3(ss)
                s3_done += 1
            pre3 = {}
            for slot in range(min(PREF, g["slots"])):
                pre3[slot] = emit_gathers_parts(
                    g, gidx_t, gd["y3_full"], slot, Y3PAD, {0})
            for slot in range(g["slots"]):
                cs = g_slot_gathers(pre3, gd["y3_full"], slot, Y3PAD)
                emit_layer3("g", g, slot, cs, gdstl_t, xT_store["gA"],
                            gohB_t, (plg, slot == 0, slot == g["slots"] - 1))
            nc.scalar.copy(out=poolacc[:B, 0:O3], in_=plg[:B, :O3])
            nc.scalar.copy(out=poolacc[:B, O3:2 * O3], in_=pls[:B, :O3])

            pool_sb = poolacc
            arsb = cp.tile([P, 4 * B], F32, tag="arsb")
            nc.vector.memset(arsb[:], 0.0)
            blocks = [("g", 0, P), ("g", 1, O3 - P), ("s", 0, P),
                      ("s", 1, O3 - P)]
            for bi, (pre, ci, m) in enumerate(blocks):
                base = 0 if pre == "g" else O3
                tps = psT.tile([P, P], F32, space="PSUM", tag="tps")
                nc.tensor.transpose(
                    out=tps[:m, :B],
                    in_=pool_sb[:B, base + ci * P:base + ci * P + m],
                    identity=ident_t[:B, :B])
                nc.scalar.copy(out=arsb[:m, bi * B:(bi + 1) * B],
                               in_=tps[:m, :B])
            nc.sync.dma_start(out=ar_in[:], in_=arsb[:])
            nc.gpsimd.collective_compute(
                "AllReduce", ALU.add,
                replica_groups=[list(range(NCORES))],
                ins=[ar_in.opt()], outs=[ar_out.opt()])
            arres = cp.tile([P, 4 * B], F32, tag="arres")
            nc.sync.dma_start(out=arres[:], in_=ar_out[:])

            zt = cp.tile([P, 5 * B], F32, tag="zt")
            nc.vector.memset(zt[:], 0.0)
            for bi, (pre, ci, m) in enumerate(blocks):
                rec = grec_t if pre == "g" else srec_t
                cor = gcorr_t if pre == "g" else scorr_t
                nc.vector.tensor_tensor(
                    out=zt[:m, bi * B:(bi + 1) * B],
                    in0=arres[:m, bi * B:(bi + 1) * B],
                    in1=rec[:m, :], op=ALU.mult)
                nc.vector.tensor_tensor(
                    out=zt[:m, bi * B:(bi + 1) * B],
                    in0=zt[:m, bi * B:(bi + 1) * B],
                    in1=cor[:m, :], op=ALU.subtract)
            nc.vector.tensor_copy(out=zt[:NF, 4 * B:5 * B], in_=pointT_t[:])

            h1 = cp.tile([P, 5 * B], F32, tag="h1")
            nc.vector.memset(h1[:], 0.0)
            for mchunk in range(5):
                m0, m1 = mchunk * P, min(600, (mchunk + 1) * P)
                hps = psT.tile([P, P], F32, space="PSUM", tag="tps")
                for k in range(5):
                    nc.tensor.matmul(
                        out=hps[:m1 - m0, :B],
                        lhsT=l1w_t[:, k * 600 + m0:k * 600 + m1],
                        rhs=zt[:, k * B:(k + 1) * B],
                        start=(k == 0), stop=(k == 4))
                nc.scalar.activation(
                    h1[:m1 - m0, mchunk * B:(mchunk + 1) * B],
                    hps[:m1 - m0, :B], AF.Relu,
                    bias=l1b_t[:m1 - m0, mchunk:mchunk + 1])
            h2 = cp.tile([P, 2 * B], F32, tag="h2")
            nc.vector.memset(h2[:], 0.0)
            for mchunk in range(2):
                m0 = mchunk * P
                hps = psT.tile([P, P], F32, space="PSUM", tag="tps")
                for k in range(5):
                    nc.tensor.matmul(
                        out=hps[:, :B],
                        lhsT=l2w_t[:, k * 256 + m0:k * 256 + m0 + P],
                        rhs=h1[:, k * B:(k + 1) * B],
                        start=(k == 0), stop=(k == 4))
                nc.scalar.activation(
                    h2[:, mchunk * B:(mchunk + 1) * B], hps[:, :B], AF.Relu,
                    bias=l2b_t[:, mchunk:mchunk + 1])
            ops = psT.tile([P, P], F32, space="PSUM", tag="tps")
            for k in range(2):
                nc.tensor.matmul(
                    out=ops[:NF, :B], lhsT=l3w_t[:, k * NF:(k + 1) * NF],
                    rhs=h2[:, k * B:(k + 1) * B],
                    start=(k == 0), stop=(k == 1))
            o3sb = cp.tile([NF, B], F32, tag="o3sb")
            nc.scalar.activation(o3sb[:], ops[:NF, :B], AF.Identity,
                                 bias=l3b_t[:NF, 0:1])
            tfin = psT.tile([P, P], F32, space="PSUM", tag="tps")
            nc.tensor.transpose(out=tfin[:B, :NF], in_=o3sb[:],
                                identity=ident_t[:NF, :NF])
            osb = cp.tile([B, NF], F32, tag="osb")
            nc.scalar.copy(out=osb[:], in_=tfin[:B, :NF])
            nc.sync.dma_start(out=out_ext[:], in_=osb[:])

            if debug:
                def dump(name, src_t, rows, cols, dtype):
                    d = nc.dram_tensor(name, [rows, cols], dtype,
                                       kind="ExternalOutput")
                    for r0 in range(0, rows, P):
                        r1 = min(rows, r0 + P)
                        bt = st_p.tile([P, cols], dtype, tag="dump")
                        nc.sync.dma_start(out=bt[:r1 - r0, :],
                                          in_=src_t[r0:r1, :])
                        nc.sync.dma_start(out=d[r0:r1, :],
                                          in_=bt[:r1 - r0, :])
                dump("dbg_xg2", gd["x2_shard"], g["S"], 2 * NF, BF16)
                dump("dbg_yg3", gd["y3_shard"], g["S"], Y3PAD, BF16)
                dump("dbg_ar", ar_out, P, 4 * B, F32)
                dump("dbg_arin", ar_in, P, 4 * B, F32)
                dbg_zt = nc.dram_tensor("dbg_zt", [P, 5 * B], F32,
                                        kind="ExternalOutput")
                zt_dump = cp.tile([P, 5 * B], F32, tag="zt_dump")
                nc.vector.tensor_copy(out=zt_dump[:], in_=zt[:])
                nc.sync.dma_start(out=dbg_zt[:], in_=zt_dump[:])

    nc.compile()
    return nc



def kernel(**inputs):
    cfg = CFG_FULL
    inputs = {k: np.asarray(v) for k, v in inputs.items()}
    meta, in_maps = host_prep(inputs, cfg)
    nc = build_program(meta)
    trace = bool(int(os.environ.get("KERNEL_TRACE", "0")))
    if trace:
        import types
        from trn_agent_boot.trn_boot import _ntff_profile_via_ctypes
        hook = _ntff_profile_via_ctypes('/opt/axon/libaxon_pjrt.so')
        mod = types.ModuleType('antenv.axon_hooks')
        mod.get_axon_ntff_profile_hook = lambda: hook
        sys.modules['antenv.axon_hooks'] = mod
    res = run_bass_kernel_spmd(nc, in_maps, list(range(NCORES)), trace=trace)
    if trace and res.exec_time_ns:
        print(f"HW exec time: {res.exec_time_ns} ns")
    return np.asarray(res.results[0]["out"], np.float32)
